# revision 19
# baseline (speedup 1.0000x reference)
"""Trainium2 Bass kernel for nn_Attention (LN -> QKV -> softmax attn -> out proj).

Sharding: 8 cores; core c handles batch b=c//4 and heads [4*(c%4), 4*(c%4)+4).
Each core computes two partial output contributions (one per head-pair stack)
of shape [1024, 2048] = (w_out slice).T @ attn_out.T; the host sums the 8
partials per batch, transposes, and adds b_out.

Device pipeline per core (bf16 matmuls, fp32 PSUM accumulate):
  A) LN stats on the DVE from a second, natural-layout copy of x:
     Sx via tensor_reduce, Sxx via tensor_tensor_reduce; rsqrt(var+eps) via
     bit-trick seed + 3 Newton steps. Row layouts (nmu, r) reach the other
     engines through one DRAM round-trip + partition-broadcast DMA per half.
  B) QKV on raw x^T with LayerNorm folded algebraically:
       qkv[t,c] = r[t] * ((x @ W')[t,c] - mu[t]*u[c])
     The -mu*u correction rides the PSUM drain as a DVE scalar_tensor_tensor;
     r is folded into BOTH q and k columns (one broadcast multiply), so the
     attention exp needs no per-partition scale. v comes out natural [t, dh].
  C) Flash-style attention without running max (matches the reference
     exactly): S^T tiles via matmul, plain exp on ScalarE (psum -> sbuf bf16),
     P@V via matmul with a ones-column appended to v so the denominator
     accumulates in the same PSUM tile; per-qp-half denominator reciprocal via
     DMA-reshape + Newton, broadcast back through DRAM.
  D) Output projection in bf16, one column-block unit at a time, interleaved
     into the h>=2 attention loops (stack 0) and the h3 tail (stack 1).
"""

import contextlib

import numpy as np

import concourse.bass as bass
import concourse.tile as tile
from concourse import bacc, mybir
from concourse import bass_utils

# Problem constants (hardcoded per contract)
B, N, DIM = 2, 2048, 1024
H, DH = 16, 64
INNER = H * DH
LN_EPS = 1e-5
ATTN_EPS = 1e-8
SCALE = DH ** -0.5

# Per-core constants
P = 128
T = N                 # tokens per core (one batch)
TT = T // P           # 16 token tiles of 128
NT4 = T // 512        # 4 token slabs of 512
KD = DIM // P         # 8 contraction tiles
HL = 4                # local heads per core
CQK = 2 * HL * DH     # 512 (q cols + k cols)
CV = HL * DH          # 256 (v cols)
GQK = CQK // P        # 4 col groups of 128
KT = T // P           # 16 key tiles of 128

f32 = mybir.dt.float32
bf16 = mybir.dt.bfloat16
FT = mybir.ActivationFunctionType
ALU = mybir.AluOpType
AXL = mybir.AxisListType

import ml_dtypes
_BF16 = np.dtype(ml_dtypes.bfloat16)

_CACHE = {}


def _hrows(h):
    """Partition slice for head h within a [128, 2, T] two-stack layout."""
    lo = 64 * (h % 2)
    return slice(lo, lo + 64), h // 2


def _build(has_v0):
    nc = bacc.Bacc("TRN2", target_bir_lowering=False, debug=False)

    xt_d = nc.dram_tensor("xt", [DIM, T], bf16, kind="ExternalInput").ap()
    xn_d = nc.dram_tensor("xn", [T, DIM], bf16, kind="ExternalInput").ap()
    wqk_d = nc.dram_tensor("wqk", [DIM, CQK], bf16, kind="ExternalInput").ap()
    wv_d = nc.dram_tensor("wv", [DIM, CV], bf16, kind="ExternalInput").ap()
    wout_d = nc.dram_tensor("wout", [2 * P, DIM], bf16, kind="ExternalInput").ap()
    uqk_d = nc.dram_tensor("uqk", [CQK], f32, kind="ExternalInput").ap()
    uv_d = nc.dram_tensor("uv", [CV], bf16, kind="ExternalInput").ap()
    v0qk_d = nc.dram_tensor("v0qk", [CQK], f32, kind="ExternalInput").ap()
    v0v_d = nc.dram_tensor("v0v", [CV], f32, kind="ExternalInput").ap()
    outp0_d = nc.dram_tensor("outp0", [DIM, T], f32, kind="ExternalOutput").ap()
    outp1_d = nc.dram_tensor("outp1", [DIM, T], f32, kind="ExternalOutput").ap()

    with tile.TileContext(nc) as tc, contextlib.ExitStack() as ctx:
        pers = ctx.enter_context(tc.tile_pool(name="pers", bufs=1))
        dram = ctx.enter_context(tc.tile_pool(name="dram", bufs=1, space="DRAM"))

        qkT = pers.tile([P, GQK, T], bf16)          # q/k transposed, heads stacked
        vaug = pers.tile([P, KT, HL, DH + 1], bf16)  # v + ones column
        outT = pers.tile([P, 2, T], bf16)           # attention output (transposed)
        wout_sb = pers.tile([P, 2, DIM], bf16)
        dnm = pers.tile([1, HL, T], f32)
        dbc = pers.tile([P, 2, T], f32)
        wones = pers.tile([1, 1], bf16)
        nc.vector.memset(wones[:], 1.0)
        wrow = pers.tile([1, 64], bf16)
        nc.vector.memset(wrow[:], 1.0)

        nmu_dram = dram.tile([1, T], f32)
        r16_dram = dram.tile([1, T], bf16)
        dnm_dram = dram.tile([HL, T], f32)
        rdn_dram = dram.tile([HL, T], f32)

        nc.vector.memset(vaug[:], 1.0)

        # ---------------- Phase A+B: stats + QKV projection ----------------
        with tc.tile_pool(name="pab", bufs=1) as pab, \
             tc.tile_pool(name="pabd", bufs=2) as pabd, \
             tc.tile_pool(name="pxn", bufs=4) as pxn, \
             tc.tile_pool(name="pgen", bufs=4, space="PSUM") as pgen, \
             tc.tile_pool(name="pgv", bufs=3, space="PSUM") as pgv:

            # --- input DMAs, priority order ---
            uqk_sb = pab.tile([P, GQK], f32)
            nc.sync.dma_start(uqk_sb[:], uqk_d.rearrange("(g p) -> p g", p=P))
            uv_bc = pab.tile([P, CV], bf16)
            nc.sync.dma_start(uv_bc[:], uv_d[None, :].to_broadcast([P, CV]))
            if has_v0:
                v0qk_sb = pab.tile([P, GQK], f32)
                nc.sync.dma_start(v0qk_sb[:], v0qk_d.rearrange("(g p) -> p g", p=P))
                v0v_bc = pab.tile([P, CV], f32)
                nc.sync.dma_start(v0v_bc[:], v0v_d[None, :].to_broadcast([P, CV]))

            def load_xn(t4):
                xn = pxn.tile([P, 4, DIM], bf16, tag="xn", name=f"xn{t4}")
                nc.sync.dma_start(
                    xn[:],
                    xn_d[t4 * 512:(t4 + 1) * 512].rearrange(
                        "(o p) c -> p o c", p=P))
                return xn

            def load_xt(t4):
                tsl = slice(t4 * 512, (t4 + 1) * 512)
                xt_t = pabd.tile([P, KD, 512], bf16, tag="xt", name=f"xt{t4}")
                nc.sync.dma_start(
                    xt_t[:], xt_d[:, tsl].rearrange("(o p) t -> p o t", p=P))
                return xt_t

            xn_t = {0: load_xn(0)}
            wqk_sb = pab.tile([P, KD, CQK], bf16)
            nc.sync.dma_start(wqk_sb[:], wqk_d.rearrange("(o p) c -> p o c", p=P))
            xn_t[1] = load_xn(1)
            xt_tiles = {0: load_xt(0)}
            xn_t[2] = load_xn(2)
            xn_t[3] = load_xn(3)
            wv_sb = pab.tile([P, KD, CV], bf16)
            nc.sync.dma_start(wv_sb[:], wv_d.rearrange("(o p) c -> p o c", p=P))
            xt_tiles[1] = load_xt(1)

            # --- stats scratch ---
            sx = pab.tile([P, TT], f32)
            sxx = pab.tile([P, TT], f32)
            sq_scr = pab.tile([P, DIM], bf16)
            mu_cc = pab.tile([P, TT], f32)
            nmu_c = pab.tile([P, TT], f32)
            ex2e = pab.tile([P, TT], f32)
            mu2 = pab.tile([P, TT], f32)
            ve = pab.tile([P, TT], f32)
            magic = pab.tile([P, TT], mybir.dt.int32)
            nc.vector.memset(magic[:], 0x5F3759DF)
            y0i = pab.tile([P, TT], mybir.dt.int32)
            t0 = pab.tile([P, TT], f32)
            r_c = pab.tile([P, TT], f32)
            r16_c = pab.tile([P, TT], bf16)
            nr_c = pab.tile([P, TT], f32)
            nmu_bc = pab.tile([P, T], f32)
            r_bc = pab.tile([P, T], bf16)

            # preload the Exp ACT table during startup idle time
            dum = pab.tile([1, 16], f32)
            nc.vector.memset(dum[:], 0.0)
            dume = pab.tile([1, 16], bf16)
            nc.scalar.activation(dume[:], dum[:], FT.Exp)

            # PE warmup during initial DMA wait
            bones = pab.tile([1, 1], bf16)
            nc.vector.memset(bones[:], 1.0)
            brow = pab.tile([1, 64], bf16)
            nc.vector.memset(brow[:], 1.0)
            warm_ps = pgen.tile([P, 512], f32, tag="qk", name="warm0")
            for _ in range(32):
                nc.tensor.matmul(warm_ps[64:65, 0:64], bones[:], brow[:],
                                 start=True, stop=True)

            def stats_sq(g):
                """Sxx for slab g on the (otherwise idle) ACT engine."""
                for o in range(4):
                    tt = g * 4 + o
                    nc.scalar.activation(
                        sq_scr[:], xn_t[g][:, o], FT.Square,
                        accum_out=sxx[:, tt:tt + 1])

            def stats_fin(g):
                """DVE Sx + rsqrt finalize for slab g -> r, nmu + broadcasts."""
                s8 = slice(g * 4, g * 4 + 4)
                for o in range(4):
                    tt = g * 4 + o
                    nc.vector.tensor_reduce(
                        sx[:, tt:tt + 1], xn_t[g][:, o], AXL.X, ALU.add)
                nc.vector.tensor_scalar(ex2e[:, s8], sxx[:, s8], 1.0 / DIM,
                                        LN_EPS, ALU.mult, ALU.add)
                nc.vector.tensor_scalar_mul(mu_cc[:, s8], sx[:, s8], 1.0 / DIM)
                nc.vector.tensor_scalar_mul(nmu_c[:, s8], sx[:, s8], -1.0 / DIM)
                nc.vector.tensor_tensor(mu2[:, s8], mu_cc[:, s8], mu_cc[:, s8],
                                        ALU.mult)
                nc.vector.scalar_tensor_tensor(ve[:, s8], mu2[:, s8], -1.0,
                                               ex2e[:, s8], ALU.mult, ALU.add)
                nc.vector.tensor_scalar(y0i[:, s8],
                                        ve[:, s8].bitcast(mybir.dt.int32), 1,
                                        None, ALU.arith_shift_right)
                nc.vector.tensor_tensor(y0i[:, s8], magic[:, s8], y0i[:, s8],
                                        ALU.subtract)
                y = y0i.bitcast(f32)
                for _ in range(3):
                    nc.vector.tensor_tensor(t0[:, s8], y[:, s8], y[:, s8],
                                            ALU.mult)
                    nc.vector.tensor_tensor(t0[:, s8], t0[:, s8], ve[:, s8],
                                            ALU.mult)
                    nc.vector.tensor_scalar(t0[:, s8], t0[:, s8], -0.5, 1.5,
                                            ALU.mult, ALU.add)
                    nc.vector.tensor_tensor(y[:, s8], y[:, s8], t0[:, s8],
                                            ALU.mult)
                nc.vector.tensor_copy(r_c[:, s8], y[:, s8])
                nc.vector.tensor_copy(r16_c[:, s8], y[:, s8])
                nc.vector.tensor_tensor(nr_c[:, s8], nmu_c[:, s8], y[:, s8],
                                        ALU.mult)
                # rows to DRAM, then partition-broadcast
                hsl = slice(g * 512, (g + 1) * 512)
                nc.sync.dma_start(
                    nmu_dram[0, hsl].rearrange("(o p) -> p o", p=P),
                    nmu_c[:, s8])
                nc.sync.dma_start(
                    r16_dram[0, hsl].rearrange("(o p) -> p o", p=P),
                    r16_c[:, s8])
                nc.sync.dma_start(nmu_bc[:, hsl],
                                  nmu_dram[0:1, hsl].to_broadcast([P, 512]))
                nc.sync.dma_start(r_bc[:, hsl],
                                  r16_dram[0:1, hsl].to_broadcast([P, 512]))

            for g in range(NT4):
                stats_sq(g)

            for t4 in range(NT4):
                tsl = slice(t4 * 512, (t4 + 1) * 512)
                if t4 + 1 < NT4:
                    xt_tiles[t4 + 1] = load_xt(t4 + 1)
                stats_fin(t4)
                xt_t = xt_tiles.pop(t4)

                # --- QKV matmuls + fused LN-fold drains for this slab ---
                for g in (2, 3, 0, 1):      # k groups first
                    ps = pgen.tile([P, 512], f32, tag="qk", name=f"qk{t4}_{g}")
                    for kt in range(KD):
                        nc.tensor.matmul(ps[:], wqk_sb[:, kt, g * P:(g + 1) * P],
                                         xt_t[:, kt],
                                         start=(kt == 0), stop=(kt == KD - 1))
                    # drain with rank-1 mean correction: (nmu * u[g]) + psum
                    nc.vector.scalar_tensor_tensor(
                        qkT[:, g, tsl], nmu_bc[:, tsl], uqk_sb[:, g:g + 1],
                        ps[:], ALU.mult, ALU.add)
                # fold r into both q and k columns
                nc.vector.tensor_tensor(
                    qkT[:, :, tsl], qkT[:, :, tsl],
                    r_bc[:, tsl][:, None, :].to_broadcast([P, GQK, 512]),
                    ALU.mult)
                if has_v0:
                    for g in range(GQK):
                        nc.vector.tensor_scalar_add(qkT[:, g, tsl],
                                                    qkT[:, g, tsl],
                                                    v0qk_sb[:, g:g + 1])

                for st in range(4):
                    tts = t4 * 4 + st
                    psv = pgv.tile([P, CV], f32, tag="v", name=f"v{t4}_{st}")
                    for kt in range(KD):
                        nc.tensor.matmul(psv[:],
                                         xt_t[:, kt, st * P:(st + 1) * P],
                                         wv_sb[:, kt],
                                         start=(kt == 0), stop=(kt == KD - 1))
                    psv3 = psv.rearrange("p (h d) -> p h d", h=HL)
                    uv3 = uv_bc.rearrange("p (h d) -> p h d", h=HL)
                    # v = r*psum (ACT drain) + (nmu*r)*u (DVE rank-1 add)
                    nc.scalar.mul(vaug[:, tts, :, 0:DH], psv3,
                                  r_c[:, tts:tts + 1])
                    nc.vector.scalar_tensor_tensor(
                        vaug[:, tts, :, 0:DH], uv3, nr_c[:, tts:tts + 1],
                        vaug[:, tts, :, 0:DH], ALU.mult, ALU.add)
                    if has_v0:
                        v03 = v0v_bc.rearrange("p (h d) -> p h d", h=HL)
                        nc.vector.tensor_tensor(vaug[:, tts, :, 0:DH],
                                                vaug[:, tts, :, 0:DH],
                                                v03, ALU.add)

            nc.sync.dma_start(wout_sb[:], wout_d.rearrange("(o p) c -> p o c", p=P))

        # ---------------- Phase C: attention (+ overlapped out-proj) -------
        with tc.tile_pool(name="pat", bufs=6) as pat, \
             tc.tile_pool(name="pdo", bufs=3) as pdo, \
             tc.tile_pool(name="psc", bufs=2, space="PSUM") as psc, \
             tc.tile_pool(name="ppv", bufs=2, space="PSUM") as ppv, \
             tc.tile_pool(name="pop", bufs=2, space="PSUM") as pop:

            # out-proj machinery: matmul+drain per (oc, t4) unit, with the
            # store DMA batched per oc row-block (4 or 2 slabs per DMA)
            op_state = {}

            def outproj_unit(stk2, od, oc, t4s, drain_eng="vector", pool=None,
                             dpool=None):
                key = (stk2, oc, t4s[0])
                n = len(t4s)
                osb = (dpool or pdo).tile([P, n, 512], f32, tag=f"osb{n}",
                                          name=f"osb{stk2}_{oc}_{t4s[0]}")
                op_state[key] = osb
                for j, t4 in enumerate(t4s):
                    tsl = slice(t4 * 512, (t4 + 1) * 512)
                    ps = (pool or pop).tile([P, 512], f32, tag="op",
                                            name=f"op{stk2}_{oc}_{t4}")
                    nc.tensor.matmul(ps[:],
                                     wout_sb[:, stk2, oc * P:(oc + 1) * P],
                                     outT[:, stk2, tsl], start=True, stop=True)
                    eng = drain_eng
                    if eng == "alt":
                        eng = "vector" if j % 2 else "scalar"
                    if eng == "vector":
                        nc.vector.tensor_copy(osb[:, j], ps[:])
                    else:
                        nc.scalar.copy(osb[:, j], ps[:])
                nc.sync.dma_start(
                    od[oc * P:(oc + 1) * P,
                       t4s[0] * 512:(t4s[-1] + 1) * 512], osb[:])

            def outproj_step(stk2, od, oc, t4, drain_eng="vector"):
                """Paced variant: one mm+drain; DMA flushes on the last slab
                of the oc group (slabs assumed visited in order)."""
                first = (stk2, oc, None) not in op_state
                if first:
                    n = 4 if stk2 == 0 else 2
                    osb = pdo.tile([P, n, 512], f32, tag=f"osb{n}",
                                   name=f"osb{stk2}_{oc}")
                    op_state[(stk2, oc, None)] = [osb, 0]
                osb, j = op_state[(stk2, oc, None)]
                tsl = slice(t4 * 512, (t4 + 1) * 512)
                ps = pop.tile([P, 512], f32, tag="op",
                              name=f"op{stk2}_{oc}_{t4}")
                nc.tensor.matmul(ps[:],
                                 wout_sb[:, stk2, oc * P:(oc + 1) * P],
                                 outT[:, stk2, tsl], start=True, stop=True)
                if drain_eng == "vector":
                    nc.vector.tensor_copy(osb[:, j], ps[:])
                else:
                    nc.scalar.copy(osb[:, j], ps[:])
                op_state[(stk2, oc, None)][1] = j + 1
                if j + 1 == osb.shape[1]:
                    base = t4 - j
                    nc.sync.dma_start(
                        od[oc * P:(oc + 1) * P,
                           base * 512:(t4 + 1) * 512], osb[:])
                    del op_state[(stk2, oc, None)]

            op0_units = [(oc, t4) for oc in range(DIM // P)
                         for t4 in range(NT4)]
            op1_units = [(oc, t4) for oc in range(DIM // P)
                         for t4 in range(2)]
            op1_late = [(oc, (2, 3)) for oc in range(DIM // P)]

            # keep the PE busy/hot across the phase transition
            wtile = psc.tile([P, 2, 512], f32, tag="sc", name="warm_at")
            for _ in range(40):
                nc.tensor.matmul(wtile[64:65, 0, 0:64], wones[:], wrow[:],
                                 start=True, stop=True)

            for h in range(HL):
                rows, stk = _hrows(h)
                for qp in range(2):
                    ps_o = [ppv.tile([DH + 1, 512], f32, tag="pv",
                                     name=f"pv{h}_{qp}_{i}") for i in range(2)]
                    etq = []
                    for kt in range(KT):
                        if h == 2 and op0_units:
                            oc_, t4_ = op0_units.pop(0)
                            outproj_step(0, outp0_d, oc_, t4_)
                        if h == 3 and qp == 1 and 4 <= kt < 12:
                            for _ in range(2):
                                if op1_units:
                                    oc_, t4_ = op1_units.pop(0)
                                    outproj_step(1, outp1_d, oc_, t4_)
                        ps_s = psc.tile([P, 2, 512], f32, tag="sc",
                                        name=f"sc{h}_{qp}_{kt}")
                        for sub in range(2):
                            qt = qp * 2 + sub
                            nc.tensor.matmul(
                                ps_s[:, sub],
                                qkT[rows, 2 + stk, kt * P:(kt + 1) * P],
                                qkT[rows, stk, qt * 512:(qt + 1) * 512],
                                start=True, stop=True)
                        et = pat.tile([P, 2, 512], bf16, tag="exp",
                                      name=f"et{h}_{qp}_{kt}")
                        nc.scalar.activation(et[:], ps_s[:], FT.Exp)
                        # PV runs two kt behind so the PE queue never blocks
                        # on the previous half's PSUM drain
                        etq.append((kt, et))
                        if len(etq) > 2:
                            pkt, pet = etq.pop(0)
                            for sub in range(2):
                                nc.tensor.matmul(ps_o[sub],
                                                 vaug[:, pkt, h, :],
                                                 pet[:, sub],
                                                 start=(pkt == 0), stop=False)
                    for pkt, pet in etq:
                        for sub in range(2):
                            nc.tensor.matmul(ps_o[sub], vaug[:, pkt, h, :],
                                             pet[:, sub],
                                             start=(pkt == 0),
                                             stop=(pkt == KT - 1))
                    for sub in range(2):
                        qt = qp * 2 + sub
                        qsl = slice(qt * 512, (qt + 1) * 512)
                        nc.vector.tensor_copy(dnm[0:1, h, qsl],
                                              ps_o[sub][DH:DH + 1])
                        nc.vector.tensor_copy(outT[rows, stk, qsl],
                                              ps_o[sub][0:DH])

                    # per-qp-half denominator reciprocal + normalize:
                    # DMA-reshape to [128, 8], bit-trick seed + 3 Newton steps
                    hsl = slice(qp * 1024, (qp + 1) * 1024)
                    nc.sync.dma_start(dnm_dram[h:h + 1, hsl], dnm[0:1, h, hsl])
                    dn2 = pat.tile([P, TT // 2], f32, tag="dn2")
                    nc.sync.dma_start(
                        dn2[:], dnm_dram[h, hsl].rearrange("(p o) -> p o", p=P))
                    rmagic = pat.tile([P, TT // 2], mybir.dt.int32, tag="rmagic")
                    nc.vector.memset(rmagic[:], 0x7EEF362E)
                    yi = pat.tile([P, TT // 2], mybir.dt.int32, tag="yi")
                    nc.vector.tensor_tensor(yi[:], rmagic[:],
                                            dn2[:].bitcast(mybir.dt.int32),
                                            ALU.subtract)
                    yf = yi.bitcast(f32)
                    tn = pat.tile([P, TT // 2], f32, tag="tn")
                    for _ in range(3):
                        nc.vector.tensor_tensor(tn[:], dn2[:], yf[:], ALU.mult)
                        nc.vector.tensor_scalar(tn[:], tn[:], -1.0, 2.0,
                                                ALU.mult, ALU.add)
                        nc.vector.tensor_tensor(yf[:], yf[:], tn[:], ALU.mult)
                    nc.sync.dma_start(
                        rdn_dram[h, hsl].rearrange("(p o) -> p o", p=P), yf[:])
                    nc.sync.dma_start(
                        dbc[rows, stk, hsl],
                        rdn_dram[h:h + 1, hsl].to_broadcast([64, 1024]))
                    nc.vector.tensor_tensor(outT[rows, stk, hsl],
                                            outT[rows, stk, hsl],
                                            dbc[rows, stk, hsl], ALU.mult)
        assert not op0_units and not op1_units and not op_state

        # ------------ Phase D: remaining output projection ----------
        with tc.tile_pool(name="pdo2", bufs=4) as pdo2, \
             tc.tile_pool(name="pop2", bufs=4, space="PSUM") as pop2:
            for i, (oc_, t4s) in enumerate(op1_late):
                outproj_unit(1, outp1_d, oc_, t4s, drain_eng="alt",
                             pool=pop2, dpool=pdo2)

    nc.compile()
    return nc


def _prep_inputs(x, ln_gamma, ln_beta, w_qkv, w_out, b_out):
    """Host-side sharding/layout prep. Returns (in_maps, has_v0)."""
    x = np.asarray(x, dtype=np.float32)
    ln_gamma = np.asarray(ln_gamma, dtype=np.float32)
    ln_beta = np.asarray(ln_beta, dtype=np.float32)
    w_qkv = np.asarray(w_qkv, dtype=np.float32)
    w_out = np.asarray(w_out, dtype=np.float32)

    wsc = w_qkv.copy()
    wsc[:, :INNER] *= SCALE                      # fold attn scale into q
    wfold = ln_gamma[:, None] * wsc              # fold LN gamma
    u = wfold.sum(axis=0)                        # [3*INNER]
    v0 = ln_beta @ wsc                           # [3*INNER]
    has_v0 = bool(np.any(v0 != 0.0))

    wq, wk, wv_all = np.split(wfold, 3, axis=1)
    uq, uk, uv_all = np.split(u, 3)
    v0q, v0k, v0v_all = np.split(v0, 3)

    in_maps = []
    for c in range(8):
        b = c // 4
        hs = (c % 4) * HL * DH
        sl = slice(hs, hs + HL * DH)
        xb = x[b]                                           # [2048, 1024]
        wqk_loc = np.concatenate([wq[:, sl], wk[:, sl]], axis=1)  # [1024, 512]
        in_maps.append({
            "xt": np.ascontiguousarray(xb.T).astype(_BF16),
            "xn": np.ascontiguousarray(xb).astype(_BF16),
            "wqk": np.ascontiguousarray(wqk_loc).astype(_BF16),
            "wv": np.ascontiguousarray(wv_all[:, sl]).astype(_BF16),
            "wout": np.ascontiguousarray(w_out[sl, :]).astype(_BF16),
            "uqk": np.concatenate([uq[sl], uk[sl]]).astype(np.float32),
            "uv": uv_all[sl].astype(_BF16),
            "v0qk": np.concatenate([v0q[sl], v0k[sl]]).astype(np.float32),
            "v0v": v0v_all[sl].astype(np.float32),
        })
    return in_maps, has_v0


def run(x, ln_gamma, ln_beta, w_qkv, w_out, b_out, trace=False, trace_kwargs=None):
    in_maps, has_v0 = _prep_inputs(x, ln_gamma, ln_beta, w_qkv, w_out, b_out)
    key = ("nc", has_v0)
    if key not in _CACHE:
        _CACHE[key] = _build(has_v0)
    nc = _CACHE[key]
    kwargs = {}
    if trace:
        kwargs = dict(trace=True, trace_cores=[0],
                      stitch_traces=False, **(trace_kwargs or {}))
    res = bass_utils.run_bass_kernel_spmd(
        nc, in_maps, core_ids=list(range(8)), **kwargs)

    b_out = np.asarray(b_out, dtype=np.float32)
    out = np.zeros((B, N, DIM), dtype=np.float32)
    for b in range(B):
        acc = np.zeros((DIM, T), dtype=np.float32)
        for c in range(4 * b, 4 * b + 4):
            acc += res.results[c]["outp0"]
            acc += res.results[c]["outp1"]
        out[b] = acc.T + b_out
    return out, res


def kernel(x, ln_gamma, ln_beta, w_qkv, w_out, b_out):
    out, _ = run(x, ln_gamma, ln_beta, w_qkv, w_out, b_out, trace=False)
    return out
